# revision 1
# baseline (speedup 1.0000x reference)
"""Trainium2 Bass kernel for nn_Decoder_64201171141372.

6-layer pre-norm transformer decoder (D=1024, H=16, F=4096, B=8, S=512).
Sharding: data-parallel over batch — each of the 8 NeuronCores computes one
batch element end-to-end; no collectives.

Device-side layout: activations are kept TRANSPOSED in SBUF as [d_part=128,
d_outer, t] so every linear layer runs as matmul(lhsT=W_tile, rhs=xT) with
the contraction dim on partitions. LayerNorm / softmax statistics (which are
per-token, i.e. per free-element) are computed with ones-matmuls and
broadcast back across partitions with rank-1 matmuls. All matmul operands
are bf16 (fp32 PSUM accumulation); the residual stream stays fp32.
"""

import math

import numpy as np
import ml_dtypes

import concourse.bass as bass
import concourse.mybir as mybir
import concourse.tile as tile
from concourse.masks import make_identity
from concourse.vector_clock import ScopedClock, VectorClock

bf16 = ml_dtypes.bfloat16
f8e4 = ml_dtypes.float8_e4m3fn
F32 = mybir.dt.float32
B16 = mybir.dt.bfloat16
F8 = mybir.dt.float8e4
I32 = mybir.dt.int32
DR = mybir.MatmulPerfMode.DoubleRow

P = 128
V, D, H, F, L = 32000, 1024, 16, 4096, 6
T = 512            # decoder seq len == encoder seq len
HD = D // H        # 64
DO = D // P        # 8
FO = F // P        # 32
TO = T // P        # 4
NSLOT = 3 * L + 1  # layernorm slots (3 per layer + final)
EPS = 1e-5
N_CORES = 8
N_PROCS = 27

AF = mybir.ActivationFunctionType
OP = mybir.AluOpType

# ---- tuning flags (overridable before build_decoder) ----
LN_PREP_ACT = False    # xbd/x2d copies on ACT (True) or DVE (False)
UB_ACT = True          # attention ub copy on ACT (True) or DVE (False)
CAUSAL_TRIM = True     # restrict causal blocks to un-masked q range
ATTN_PAIR = False      # pair-adjacent score matmuls
FILLERS = True         # interleave cross-K/V into self-attention
LN2_FILL = 2           # cross-K/V pieces held back to fill the LN2 stats bubble
ET_BUFS = 3            # expT pipeline depth
RB_BUFS = 2            # rdb/ub pipeline depth

# fp8 (e4m3) config: q/k/v projections tolerate fp8 (softmax damping / small
# contribution); o/w1/w2 measurably do not (rel err budget is 2e-2).
WS = 64.0              # fp8 weight pre-scale (keeps 0.02-scale weights normal)
FP8_QKV = True         # q,k,v projections (self+cross) via fp8 DoubleRow
FP8_CROSS_AV = True    # cross-attention AV via fp8 DoubleRow
FP8_LN_STATS = True    # LN stats matmuls via fp8 DoubleRow
FP8_W = ("saq", "sak", "sav", "caq", "cak", "cav")


class _TC(tile.TileContext):
    """TileContext whose exit drain splits sem waits one per instruction.

    The walrus build in this container only encodes a single sync-wait on
    CTRL-class (Drain/NoOp) instructions; the stock tile exit aggregates one
    wait per logical proc onto one Drain and dies in codegen with "Too many
    sync wait commands". Emit one single-wait NOP per proc on the SP queue
    instead; SP program order then makes the final drain safe with no waits.
    """

    def _drain_and_barrier(self, tick_clock, wait_clock):
        gc = tick_clock.global_clock
        for p in range(N_PROCS):
            t = gc[p]
            if t:
                nop = self.nc.sync.nop(nofuse=True)
                pc = VectorClock([t if q == p else 0 for q in range(N_PROCS)])
                wait_clock.add_sem_waits(nop.ins, ScopedClock({None: pc}))
        self.nc.sync.drain()
        self.nc.all_engine_barrier()
        assert self.sems is not None
        popped = self.nc._tile_sem_poison_stack.pop()
        assert popped is self._sem_poison
        self.nc.clear_and_free_semaphores(list(self.sems.allocated().values()))
        self.nc.all_engine_barrier()


def _split_sync_waits(nc, max_waits=1):
    """Walrus in this container encodes at most one sync-wait per instruction.

    Tile's wait assigner attaches one wait per depended-on logical proc; move
    the extras onto single-wait NoOps inserted just before the instruction on
    the same engine queue (program order preserves the sync semantics).
    """
    n_added = 0
    for f in nc.m.functions:
        for bb in f.blocks:
            insts = bb.instructions
            new_list = []
            for ins in insts:
                si = getattr(ins, "sync_info", None)
                waits = list(si.on_wait) if si is not None and si.on_wait else []
                if len(waits) > max_waits:
                    for w in waits[:-max_waits]:
                        nop = mybir.InstNoOp(
                            name=f"I-wsplit{n_added}", ins=[], outs=[])
                        nop.engine = ins.engine
                        nop.sync_info = mybir.SyncInfo(on_wait=[w], on_update=[])
                        new_list.append(nop)
                        n_added += 1
                    ins.sync_info = mybir.SyncInfo(
                        on_wait=waits[-max_waits:], on_update=list(si.on_update))
                new_list.append(ins)
            if n_added:
                insts[:] = new_list
    return n_added


# ---------------------------------------------------------------- packing ---

def _wpack_offsets():
    """Column offsets into the bf16 pack and the fp8 pack.

    Each weight W[din, dout] is stored as [128, din//128, dout] flattened on
    the free axis; value at [p, ko, n] = W[ko*128 + p, n]. Returns
    offs[name] = (kind, col, ko, n) with kind in {"w16", "w8"}, plus the two
    pack widths.
    """
    fp8 = set(FP8_W) if FP8_QKV else set()
    offs = {}
    c16 = 0
    c8 = 0

    def add(name, nm, ko, n):
        nonlocal c16, c8
        if nm in fp8:
            offs[name] = ("w8", c8, ko, n)
            c8 += ko * n
        else:
            offs[name] = ("w16", c16, ko, n)
            c16 += ko * n

    for l in range(L):
        for nm, ko, n in [
            ("saq", 8, 1024), ("sak", 8, 1024), ("sav", 8, 1024), ("sao", 8, 1024),
            ("caq", 8, 1024), ("cak", 8, 1024), ("cav", 8, 1024), ("cao", 8, 1024),
            ("w1", 8, 4096), ("w2", 32, 1024),
        ]:
            add(f"{nm}{l}", nm, ko, n)
    return offs, c16, c8


def _bpack_offsets():
    """Column offsets into bpack [128, cols] f32: bias b[d] at [d%128, off + d//128]."""
    offs = {}
    c = 0

    def add(name, w):
        nonlocal c
        offs[name] = c
        c += w

    for l in range(L):
        for nm, w in [
            ("sabq", 8), ("sabk", 8), ("sabv", 8), ("sabo", 8),
            ("cabq", 8), ("cabk", 8), ("cabv", 8), ("cabo", 8),
            ("b1", 32), ("b2", 8),
        ]:
            add(f"{nm}{l}", w)
    return offs, c


def _col_major(w):
    """[din, n] -> [128, din//128, n] with [p, ko, n] = w[ko*128+p, n]."""
    din, n = w.shape
    return np.ascontiguousarray(w.reshape(din // P, P, n).transpose(1, 0, 2))


def _part_cols(b):
    """[d] -> [128, d//128] with [p, o] = b[o*128+p]."""
    return np.ascontiguousarray(b.reshape(-1, P).T)


def prep_inputs(inputs):
    """Host-side packing: returns (in_maps list for 8 cores)."""
    enc = np.asarray(inputs["encoder_output"], np.float32)       # [8, 512, 1024]
    dec = np.asarray(inputs["decoder_input"]).astype(np.int32)   # [8, 512]
    table = np.ascontiguousarray(np.asarray(inputs["embed_table"], np.float32))
    sa_w = np.asarray(inputs["sa_w"], np.float32)
    sa_b = np.asarray(inputs["sa_b"], np.float32)
    ca_w = np.asarray(inputs["ca_w"], np.float32)
    ca_b = np.asarray(inputs["ca_b"], np.float32)
    w1 = np.asarray(inputs["ffn_w1"], np.float32)
    b1 = np.asarray(inputs["ffn_b1"], np.float32)
    w2 = np.asarray(inputs["ffn_w2"], np.float32)
    b2 = np.asarray(inputs["ffn_b2"], np.float32)
    ln_g = np.asarray(inputs["ln_g"], np.float32)
    ln_b = np.asarray(inputs["ln_b"], np.float32)
    fin_g = np.asarray(inputs["final_g"], np.float32)
    fin_b = np.asarray(inputs["final_b"], np.float32)

    woffs, wcols16, wcols8 = _wpack_offsets()
    wpack = np.empty((P, wcols16), dtype=bf16)
    wpack8 = np.empty((P, max(wcols8, 1)), dtype=f8e4)

    def put(name, w):
        kind, off, ko, n = woffs[name]
        flat = _col_major(w).reshape(P, -1)
        if kind == "w8":
            wpack8[:, off:off + ko * n] = (flat * WS).astype(f8e4)
        else:
            wpack[:, off:off + ko * n] = flat.astype(bf16)

    for l in range(L):
        for j, nm in enumerate(["saq", "sak", "sav", "sao"]):
            put(f"{nm}{l}", sa_w[l, j])
        for j, nm in enumerate(["caq", "cak", "cav", "cao"]):
            put(f"{nm}{l}", ca_w[l, j])
        put(f"w1{l}", w1[l])
        put(f"w2{l}", w2[l])

    boffs, bcols = _bpack_offsets()
    bpack = np.zeros((P, bcols), dtype=np.float32)
    for l in range(L):
        for j, nm in enumerate(["sabq", "sabk", "sabv", "sabo"]):
            bpack[:, boffs[f"{nm}{l}"]:boffs[f"{nm}{l}"] + 8] = _part_cols(sa_b[l, j])
        for j, nm in enumerate(["cabq", "cabk", "cabv", "cabo"]):
            bpack[:, boffs[f"{nm}{l}"]:boffs[f"{nm}{l}"] + 8] = _part_cols(ca_b[l, j])
        bpack[:, boffs[f"b1{l}"]:boffs[f"b1{l}"] + 32] = _part_cols(b1[l])
        bpack[:, boffs[f"b2{l}"]:boffs[f"b2{l}"] + 8] = _part_cols(b2[l])

    gln = np.empty((2, NSLOT, D), dtype=bf16)
    for l in range(L):
        for s in range(3):
            gln[0, 3 * l + s] = ln_g[l, s].astype(bf16)
            gln[1, 3 * l + s] = ln_b[l, s].astype(bf16)
    gln[0, NSLOT - 1] = fin_g.astype(bf16)
    gln[1, NSLOT - 1] = fin_b.astype(bf16)

    # positional encoding, transposed layout [128, 8, 512] fp32
    pos = np.arange(T, dtype=np.float32)[:, None]
    div = np.exp(np.arange(0, D, 2, dtype=np.float32) * (-math.log(10000.0) / D))
    pe = np.zeros((T, D), dtype=np.float32)
    pe[:, 0::2] = np.sin(pos * div)
    pe[:, 1::2] = np.cos(pos * div)
    peT = np.ascontiguousarray(pe.T.reshape(DO, P, T).transpose(1, 0, 2))

    # with CAUSAL_TRIM only the diagonal [P, P] block is ever masked, and it
    # is the same lower-triangular(<=) pattern for every kto: [p, q] = p <= q
    cmask = (np.arange(P)[:, None] <= np.arange(P)[None, :]).astype(bf16)

    enc_dt = f8e4 if FP8_QKV else bf16
    in_maps = []
    for c in range(N_CORES):
        encT = np.ascontiguousarray(
            enc[c].T.reshape(DO, P, T).transpose(1, 0, 2)).astype(enc_dt)
        in_maps.append({
            "wpack": wpack,
            "wpack8": wpack8,
            "bpack": bpack,
            "gln": gln,
            "table": table,
            "idx": dec[c].copy(),
            "encT": encT,
            "peT": peT,
            "cmask": cmask,
        })
    return in_maps


def unshard(results):
    """Per-core outT [128, 8, 512] -> full [8, 512, 1024] fp32."""
    out = np.empty((N_CORES, T, D), dtype=np.float32)
    for c in range(N_CORES):
        arr = results[c]["out"]                       # [dp, do, t]
        out[c] = arr.transpose(2, 1, 0).reshape(T, D)  # [t, do*128+dp]
    return out


# ----------------------------------------------------------------- device ---

def build_decoder(repeat: int = 1):
    nc = bass.Bass(trn_type="TRN2")
    woffs, wcols16, wcols8 = _wpack_offsets()
    boffs, bcols = _bpack_offsets()
    enc_dt = F8 if FP8_QKV else B16

    w_dram = nc.dram_tensor("wpack", [P, wcols16], B16, kind="ExternalInput")
    w8_dram = nc.dram_tensor("wpack8", [P, max(wcols8, 1)], F8,
                             kind="ExternalInput")
    b_dram = nc.dram_tensor("bpack", [P, bcols], F32, kind="ExternalInput")
    gln_dram = nc.dram_tensor("gln", [2, NSLOT, D], B16, kind="ExternalInput")
    table = nc.dram_tensor("table", [V, D], F32, kind="ExternalInput")
    idx_dram = nc.dram_tensor("idx", [T], I32, kind="ExternalInput")
    enc_dram = nc.dram_tensor("encT", [P, DO, T], enc_dt, kind="ExternalInput")
    pe_dram = nc.dram_tensor("peT", [P, DO, T], F32, kind="ExternalInput")
    cm_dram = nc.dram_tensor("cmask", [P, P], B16, kind="ExternalInput")
    out_dram = nc.dram_tensor("out", [P, DO, T], F32, kind="ExternalOutput")

    with _TC(nc) as tc:
        with tc.tile_pool(name="pers", bufs=1) as pers, \
             tc.tile_pool(name="wp", bufs=2) as wp, \
             tc.tile_pool(name="act", bufs=1) as act, \
             tc.tile_pool(name="sc", bufs=2) as scp, \
             tc.tile_pool(name="sm", bufs=1) as sm, \
             tc.tile_pool(name="ps", bufs=8, space="PSUM") as psp:

            # ---- persistent state ----
            x = pers.tile([P, DO, T], F32)       # residual stream (transposed)
            ones = pers.tile([P, T], B16)
            ones8 = pers.tile([P, 2, 16], F8)    # fp8 ones; 16B k-stride for dual-fp8 ldweights
            ident = pers.tile([P, P], F32)
            bias_sb = pers.tile([P, bcols], F32)
            enc_sb = pers.tile([P, DO, T], enc_dt)
            cm_sb = pers.tile([P, P], B16)
            idx_sb = pers.tile([P, TO], I32)

            zcol = pers.tile([P, 1], F32)    # zero bias column for ACT ops
            epsc = pers.tile([1, 1], F32)    # eps bias for the LN sqrt
            sA = pers.tile([1, T], B16)      # LN scale row (bf16 rhs for A-mm)
            sB = pers.tile([2, T], B16)      # LN shift row + ones row (B-mm rhs)

            nc.vector.memset(ones[:], 1.0)
            nc.vector.memset(ones8[:], 1.0)
            nc.vector.memset(sB[:], 1.0)
            nc.vector.memset(zcol[:], 0.0)
            nc.vector.memset(epsc[:], EPS)
            make_identity(nc, ident[:])
            nc.sync.dma_start(bias_sb[:], b_dram[:])
            nc.sync.dma_start(enc_sb[:], enc_dram[:])
            nc.sync.dma_start(cm_sb[:], cm_dram[:])
            nc.sync.dma_start(idx_sb[:], idx_dram.rearrange("(ti p) -> p ti", p=P))

            def psum(tag="ps"):
                return psp.tile([P, T], F32, tag=tag, name="pt")

            def load_w(name):
                kind, off, ko, n = woffs[name]
                if n == 4096:  # w1: select a 1024-wide column group q later
                    raise AssertionError("use load_w1")
                if kind == "w8":
                    wt = wp.tile([P, 8, 1024], F8, tag="w8")
                    src = w8_dram[:, off:off + ko * n].rearrange(
                        "p (o n) -> p o n", o=ko)
                else:
                    wt = wp.tile([P, 8, 1024], B16, tag="w")
                    src = w_dram[:, off:off + ko * n].rearrange(
                        "p (o n) -> p o n", o=ko)
                nc.sync.dma_start(wt[:], src)
                return wt, kind

            def load_w1(l, q):
                kind, off, ko, n = woffs[f"w1{l}"]
                wt = wp.tile([P, 8, 1024], B16, tag="w")
                src = w_dram[:, off:off + ko * n].rearrange("p (o n) -> p o n", o=ko)
                nc.sync.dma_start(wt[:], src[:, :, q * 1024:(q + 1) * 1024])
                return wt

            def load_w2(l, q):
                kind, off, ko, n = woffs[f"w2{l}"]
                wt = wp.tile([P, 8, 1024], B16, tag="w")
                src = w_dram[:, off + q * 8192: off + (q + 1) * 8192]
                nc.sync.dma_start(wt[:], src.rearrange("p (o n) -> p o n", o=8))
                return wt

            def mm_col(pq, wt, kind, do, rhs_t, q0=0):
                """Accumulate one output column tile: psum += W.T @ rhs."""
                if kind == "w8":
                    for kp in range(DO // 2):
                        nc.tensor.matmul(
                            pq[:, q0:],
                            lhsT=wt[:, 2 * kp:2 * kp + 2, do * P:(do + 1) * P],
                            rhs=rhs_t[:, 2 * kp:2 * kp + 2, q0:],
                            perf_mode=DR,
                            start=(kp == 0), stop=(kp == DO // 2 - 1))
                else:
                    for ko in range(DO):
                        nc.tensor.matmul(
                            pq[:, q0:], lhsT=wt[:, ko, do * P:(do + 1) * P],
                            rhs=rhs_t[:, ko, q0:],
                            start=(ko == 0), stop=(ko == DO - 1))

            # ---------------- layer building blocks ----------------
            def layer_norm(slot, out_t, final=False, fillers=()):
                """out_t[:, do, :] = LN(x) using gln[:, slot]; out dtype = out_t's."""
                s1 = psum()
                s2 = psum()
                if FP8_LN_STATS:
                    # pair-granular fp8 copies + DoubleRow stat matmuls.
                    # x2d holds x^2/8 (keeps squares in e4m3 range).
                    xbds, x2ds = [], []
                    for kp in range(DO // 2):
                        xbd = scp.tile([P, 2, T], F8, tag="xbd", bufs=4,
                                       name="xbd")
                        nc.vector.tensor_copy(
                            xbd[:], x[:, 2 * kp:2 * kp + 2, :])
                        xbds.append(xbd)
                    for kp in range(DO // 2):
                        x2d = scp.tile([P, 2, T], F8, tag="x2d", bufs=4,
                                       name="x2d")
                        nc.vector.scalar_tensor_tensor(
                            x2d[:], x[:, 2 * kp:2 * kp + 2, :], 0.125,
                            x[:, 2 * kp:2 * kp + 2, :],
                            op0=OP.mult, op1=OP.mult)
                        x2ds.append(x2d)
                    for kp in range(DO // 2):
                        nc.tensor.matmul(s1[0:2, :], lhsT=ones8[:, :, 0:2],
                                         rhs=xbds[kp][:], perf_mode=DR,
                                         start=(kp == 0),
                                         stop=(kp == DO // 2 - 1))
                        nc.tensor.matmul(s2[0:2, :], lhsT=ones8[:, :, 0:2],
                                         rhs=x2ds[kp][:], perf_mode=DR,
                                         start=(kp == 0),
                                         stop=(kp == DO // 2 - 1))
                    s2scale = 8.0
                else:
                    # group same-function ACT ops to avoid table thrash
                    for g in range(2):
                        xbds, x2ds = [], []
                        for dl in range(4):
                            xbd = scp.tile([P, T], B16, tag="xbd", bufs=4,
                                           name="xbd")
                            if LN_PREP_ACT:
                                nc.scalar.copy(xbd[:], x[:, g * 4 + dl, :])
                            else:
                                nc.vector.tensor_copy(xbd[:], x[:, g * 4 + dl, :])
                            xbds.append(xbd)
                        for dl in range(4):
                            x2d = scp.tile([P, T], B16, tag="x2d", bufs=4,
                                           name="x2d")
                            if LN_PREP_ACT:
                                nc.scalar.square(x2d[:], x[:, g * 4 + dl, :])
                            else:
                                nc.vector.tensor_tensor(
                                    x2d[:], x[:, g * 4 + dl, :],
                                    x[:, g * 4 + dl, :], op=OP.mult)
                            x2ds.append(x2d)
                        for dl in range(4):
                            do = g * 4 + dl
                            nc.tensor.matmul(s1[0:1, :], lhsT=ones[:, 0:1],
                                             rhs=xbds[dl][:],
                                             start=(do == 0), stop=(do == DO - 1))
                            nc.tensor.matmul(s2[0:1, :], lhsT=ones[:, 0:1],
                                             rhs=x2ds[dl][:],
                                             start=(do == 0), stop=(do == DO - 1))
                    s2scale = 1.0
                for f in fillers:
                    f()
                m = sm.tile([1, T], F32, tag="m")
                t1 = sm.tile([1, T], F32, tag="t1")
                t2 = sm.tile([1, T], F32, tag="t2")
                # mean; m^2; var = s2*s2scale/D - m^2; rstd = 1/sqrt(var+eps)
                nc.vector.tensor_scalar_mul(m[:], s1[0:1, :], 1.0 / D)
                nc.vector.tensor_tensor(t1[:], m[:], m[:], op=OP.mult)
                nc.vector.scalar_tensor_tensor(t2[:], s2[0:1, :], s2scale / D,
                                               t1[:],
                                               op0=OP.mult, op1=OP.subtract)
                nc.scalar.activation(t1[:], t2[:], AF.Sqrt, bias=epsc[:])
                nc.vector.reciprocal(t2[:], t1[:])
                nc.vector.tensor_copy(sA[:], t2[:])
                nc.vector.scalar_tensor_tensor(sB[0:1, :], m[:], -1.0, t2[:],
                                               op0=OP.mult, op1=OP.mult)
                gl = sm.tile([2, 1, D], B16, tag="gl")
                nc.sync.dma_start(gl[:], gln_dram[:, slot, :][:, None, :])
                for do in range(DO):
                    A = psum()
                    Bp = psum()
                    nc.tensor.matmul(A[:], lhsT=gl[0:1, 0, do * P:(do + 1) * P],
                                     rhs=sA[:], start=True, stop=True)
                    nc.tensor.matmul(Bp[:], lhsT=gl[0:2, 0, do * P:(do + 1) * P],
                                     rhs=sB[:], start=True, stop=True)
                    tmp = scp.tile([P, T], B16, tag="tmp")
                    nc.vector.tensor_tensor(tmp[:], x[:, do, :], A[:], op=OP.mult)
                    nc.vector.tensor_tensor(out_t[:, do, :], tmp[:], Bp[:], op=OP.add)

            def proj_T(wname, bname, rhs_t, out_t, ko_outer=False):
                """out_t[dout, t] (transposed layout, bf16) = W.T @ rhs + b.

                ko_outer: iterate the contraction dim outermost (groups of 4
                output tiles) so the first matmuls only need rhs slice ko=0 —
                used for the first consumer after a layernorm, whose apply
                produces rhs slices incrementally."""
                wt, kind = load_w(wname)
                boff = boffs[bname]
                osc = 1.0 / WS if kind == "w8" else 1.0
                if not ko_outer:
                    for do in range(DO):
                        pq = psum()
                        mm_col(pq, wt, kind, do, rhs_t)
                        nc.scalar.activation(out_t[:, do, :], pq[:], AF.Identity,
                                             bias=bias_sb[:, boff + do:boff + do + 1],
                                             scale=osc)
                else:
                    for grp in range(2):
                        pqs = [psum() for _ in range(4)]
                        if kind == "w8":
                            for kp in range(DO // 2):
                                for dl in range(4):
                                    do = grp * 4 + dl
                                    nc.tensor.matmul(
                                        pqs[dl][:],
                                        lhsT=wt[:, 2 * kp:2 * kp + 2,
                                                do * P:(do + 1) * P],
                                        rhs=rhs_t[:, 2 * kp:2 * kp + 2, :],
                                        perf_mode=DR,
                                        start=(kp == 0), stop=(kp == DO // 2 - 1))
                        else:
                            for ko in range(DO):
                                for dl in range(4):
                                    do = grp * 4 + dl
                                    nc.tensor.matmul(
                                        pqs[dl][:], lhsT=wt[:, ko, do * P:(do + 1) * P],
                                        rhs=rhs_t[:, ko, :],
                                        start=(ko == 0), stop=(ko == DO - 1))
                        for dl in range(4):
                            do = grp * 4 + dl
                            nc.scalar.activation(
                                out_t[:, do, :], pqs[dl][:], AF.Identity,
                                bias=bias_sb[:, boff + do:boff + do + 1],
                                scale=osc)

            def proj_V(wname, rhs_t, v65_t):
                """v65_t[:, to, h, 0:64] = (rhs.T @ Wv) in natural [t, dout] layout."""
                wt, kind = load_w(wname)
                osc = 1.0 / WS if kind == "w8" else 1.0
                for to in range(TO):
                    for nh in range(2):
                        pv = psum()
                        if kind == "w8":
                            for kp in range(DO // 2):
                                nc.tensor.matmul(
                                    pv[:],
                                    lhsT=rhs_t[:, 2 * kp:2 * kp + 2,
                                               to * P:(to + 1) * P],
                                    rhs=wt[:, 2 * kp:2 * kp + 2,
                                           nh * 512:(nh + 1) * 512],
                                    perf_mode=DR,
                                    start=(kp == 0), stop=(kp == DO // 2 - 1))
                        else:
                            for ko in range(DO):
                                nc.tensor.matmul(
                                    pv[:], lhsT=rhs_t[:, ko, to * P:(to + 1) * P],
                                    rhs=wt[:, ko, nh * 512:(nh + 1) * 512],
                                    start=(ko == 0), stop=(ko == DO - 1))
                        if kind == "w8":
                            nc.vector.tensor_scalar_mul(
                                v65_t[:, to, nh * 8:(nh + 1) * 8, 0:64],
                                pv.rearrange("p (h d) -> p h d", d=HD), osc)
                        else:
                            nc.vector.tensor_copy(
                                v65_t[:, to, nh * 8:(nh + 1) * 8, 0:64],
                                pv.rearrange("p (h d) -> p h d", d=HD))

            def attention(qt_t, kt_t, v65_t, out_att, causal, bvname,
                          fillers=(), dr=False):
                """Pipelined per-head (or per-pair) softmax attention.

                dr: et in fp8 + DoubleRow AV (cross-attention only — needs
                the full untrimmed q range per k-tile)."""
                bvoff = boffs[bvname]
                fillers = list(fillers)
                trim = causal and CAUSAL_TRIM
                et_dt = F8 if dr else B16
                assert not (dr and (ATTN_PAIR or causal))

                def q0_of(kto):
                    return kto * P if trim else 0

                def scores_exp(h):
                    """scores + exp (+mask) for one head; returns et."""
                    base = (h % 2) * HD
                    doh = h // 2
                    scs = []
                    for kto in range(TO):
                        q0 = q0_of(kto)
                        sc = psum()
                        nc.tensor.matmul(
                            sc[:, q0:],
                            lhsT=kt_t[base:base + HD, doh, kto * P:(kto + 1) * P],
                            rhs=qt_t[base:base + HD, doh, q0:],
                            start=True, stop=True)
                        scs.append(sc)
                    et = scp.tile([P, TO, T], et_dt, tag="expT", bufs=ET_BUFS,
                                  name="et")
                    for kto in range(TO):
                        q0 = q0_of(kto)
                        nc.scalar.activation(et[:, kto, q0:], scs[kto][:, q0:],
                                             AF.Exp, bias=zcol[:],
                                             scale=1.0 / math.sqrt(HD))
                        if causal:
                            qe = q0 + P if trim else T
                            nc.vector.tensor_tensor(
                                et[:, kto, q0:qe], et[:, kto, q0:qe],
                                cm_sb[:, 0:qe - q0], op=OP.mult)
                    return et

                def pair_scores_exp(pr):
                    """scores + exp for a head pair, score mms pair-adjacent."""
                    et = scp.tile([P, TO, 2, T], B16, tag="expT", bufs=2,
                                  name="et")
                    for kto in range(TO):
                        q0 = q0_of(kto)
                        scs = []
                        for e in range(2):
                            sc = psum()
                            nc.tensor.matmul(
                                sc[:, q0:],
                                lhsT=kt_t[e * HD:(e + 1) * HD, pr,
                                          kto * P:(kto + 1) * P],
                                rhs=qt_t[e * HD:(e + 1) * HD, pr, q0:],
                                start=True, stop=True)
                            scs.append(sc)
                        for e in range(2):
                            nc.scalar.activation(et[:, kto, e, q0:],
                                                 scs[e][:, q0:], AF.Exp,
                                                 bias=zcol[:],
                                                 scale=1.0 / math.sqrt(HD))
                        if causal:
                            qe = q0 + P if trim else T
                            for e in range(2):
                                nc.vector.tensor_tensor(
                                    et[:, kto, e, q0:qe], et[:, kto, e, q0:qe],
                                    cm_sb[:, 0:qe - q0], op=OP.mult)
                    return et

                def emit_ud(h, et_sl, et_tile=None):
                    ud = psum()
                    if dr:
                        # fp8 DoubleRow over k-tile pairs (full q range)
                        for kp in range(TO // 2):
                            nc.tensor.matmul(
                                ud[0:HD + 1, :],
                                lhsT=v65_t[:, 2 * kp:2 * kp + 2, h, :],
                                rhs=et_tile[:, 2 * kp:2 * kp + 2, :],
                                perf_mode=DR,
                                start=(kp == 0), stop=(kp == TO // 2 - 1))
                        return ud
                    for kto in range(TO):
                        q0 = q0_of(kto)
                        nc.tensor.matmul(ud[0:HD + 1, q0:],
                                         lhsT=v65_t[:, kto, h, :],
                                         rhs=et_sl(kto)[:, q0:],
                                         start=(kto == 0), stop=(kto == TO - 1))
                    return ud

                def emit_recip_ub(ud):
                    rdb = scp.tile([P, T], B16, tag="rdb", bufs=RB_BUFS, name="rdb")
                    with nc.allow_low_precision("softmax denom recip bf16"):
                        nc.vector.reciprocal(rdb[HD:HD + 1, :],
                                             ud[HD:HD + 1, :])
                    ub = scp.tile([P, T], B16, tag="ub", bufs=RB_BUFS, name="ub")
                    if UB_ACT:
                        nc.scalar.activation(ub[0:HD, :], ud[0:HD, :], AF.Copy)
                    else:
                        nc.vector.tensor_copy(ub[0:HD, :], ud[0:HD, :])
                    return rdb, ub

                def emit_norm(h, ub, rdb):
                    base = (h % 2) * HD
                    doh = h // 2
                    rb = psum()
                    nc.tensor.matmul(rb[0:HD, :], lhsT=ones[HD:HD + 1, 0:HD],
                                     rhs=rdb[HD:HD + 1, :], start=True, stop=True)
                    sl = out_att[base:base + HD, doh, :]
                    nc.vector.tensor_tensor(sl, ub[0:HD, :], rb[0:HD, :],
                                            op=OP.mult)
                    nc.vector.tensor_scalar_add(
                        sl, sl, bias_sb[base:base + HD, bvoff + doh:bvoff + doh + 1])

                if not ATTN_PAIR:
                    et = scores_exp(0)
                    pending = None
                    for h in range(H):
                        if h + 1 < H:
                            net = scores_exp(h + 1)
                        cur = et
                        ud = emit_ud(h, lambda kto: cur[:, kto, :], et_tile=cur)
                        rdb, ub = emit_recip_ub(ud)
                        if fillers:
                            fillers.pop(0)()
                        if pending is not None:
                            emit_norm(*pending)
                        pending = (h, ub, rdb)
                        if h + 1 < H:
                            et = net
                    emit_norm(*pending)
                else:
                    et = pair_scores_exp(0)
                    for pr in range(H // 2):
                        cur = et
                        items = []
                        for e in range(2):
                            ud = emit_ud(2 * pr + e,
                                         lambda kto, e=e: cur[:, kto, e, :])
                            rdb, ub = emit_recip_ub(ud)
                            items.append((2 * pr + e, ub, rdb))
                        if fillers:
                            fillers.pop(0)()
                        if fillers:
                            fillers.pop(0)()
                        for it in items:
                            emit_norm(*it)
                        if pr + 1 < H // 2:
                            et = pair_scores_exp(pr + 1)
                for f in fillers:
                    f()

            def proj_O(wname, bname, rhs_att):
                """x += W.T @ att + b (residual update)."""
                wt, kind = load_w(wname)
                assert kind == "w16", "o-projection stays bf16 (error budget)"
                boff = boffs[bname]
                for do in range(DO):
                    po = psum()
                    for ko in range(DO):
                        nc.tensor.matmul(po[:], lhsT=wt[:, ko, do * P:(do + 1) * P],
                                         rhs=rhs_att[:, ko, :],
                                         start=(ko == 0), stop=(ko == DO - 1))
                    nc.vector.scalar_tensor_tensor(
                        x[:, do, :], po[:], bias_sb[:, boff + do:boff + do + 1],
                        x[:, do, :], op0=OP.add, op1=OP.add)

            # ---------------- full forward pass ----------------
            def body():
                # embedding: gather rows, transpose via PE, scale + pos-enc
                for ti in range(TO):
                    x0 = scp.tile([P, D], F32, tag="x0", bufs=1)
                    nc.gpsimd.indirect_dma_start(
                        out=x0[:], out_offset=None, in_=table[:],
                        in_offset=bass.IndirectOffsetOnAxis(
                            ap=idx_sb[:, ti:ti + 1], axis=0))
                    for do in range(DO):
                        pst = psum()
                        nc.tensor.transpose(pst[:, 0:P], x0[:, do * P:(do + 1) * P],
                                            ident[:])
                        pe_part = scp.tile([P, P], F32, tag="pe")
                        nc.sync.dma_start(pe_part[:],
                                          pe_dram[:, do, ti * P:(ti + 1) * P])
                        nc.vector.scalar_tensor_tensor(
                            x[:, do, ti * P:(ti + 1) * P], pst[:, 0:P],
                            math.sqrt(D), pe_part[:], op0=OP.mult, op1=OP.add)

                hdt = F8 if FP8_QKV else B16
                vedt = F8 if FP8_CROSS_AV else B16
                hb = act.tile([P, DO, T], hdt, tag="hb")      # attention LNs
                hbf = act.tile([P, DO, T], B16, tag="hbf")    # FFN LN (bf16)
                qt = act.tile([P, DO, T], B16, tag="qt")
                kt = act.tile([P, DO, T], B16, tag="kt", bufs=2)
                att = act.tile([P, DO, T], B16, tag="att")

                for l in range(L):
                    uT = act.tile([P, FO, T], B16, tag="uT", name="uT")
                    # ---- self attention ----
                    layer_norm(3 * l + 0, hb)
                    proj_T(f"saq{l}", f"sabq{l}", hb, qt)
                    proj_T(f"sak{l}", f"sabk{l}", hb, kt)
                    v65 = act.tile([P, TO, H, HD + 1], B16, tag="v65", bufs=2)
                    nc.vector.memset(v65[:, :, :, HD:HD + 1], 1.0)
                    proj_V(f"sav{l}", hb, v65)

                    # cross-attn K/V only depend on the encoder: emit them as
                    # fillers between self-attention heads to keep PE busy.
                    kte = act.tile([P, DO, T], B16, tag="kt", bufs=2)
                    v65e = act.tile([P, TO, H, HD + 1], vedt, tag="v65",
                                    bufs=2)
                    wke, kindk = load_w(f"cak{l}")
                    wve, kindv = load_w(f"cav{l}")
                    ksc = 1.0 / WS if kindk == "w8" else 1.0
                    vsc = 1.0 / WS if kindv == "w8" else 1.0
                    kboff = boffs[f"cabk{l}"]
                    fillers = []

                    def mk_kenc(do, wke=wke, kindk=kindk, kte=kte,
                                kboff=kboff, ksc=ksc):
                        def fill():
                            pq = psum()
                            mm_col(pq, wke, kindk, do, enc_sb)
                            nc.scalar.activation(
                                kte[:, do, :], pq[:], AF.Identity,
                                bias=bias_sb[:, kboff + do:kboff + do + 1],
                                scale=ksc)
                        return fill

                    def mk_venc(to, nh, wve=wve, kindv=kindv, v65e=v65e,
                                vsc=vsc):
                        def fill():
                            if to == 0 and nh == 0:
                                nc.vector.memset(v65e[:, :, :, HD:HD + 1], 1.0)
                            pv = psum()
                            if kindv == "w8":
                                for kp in range(DO // 2):
                                    nc.tensor.matmul(
                                        pv[:],
                                        lhsT=enc_sb[:, 2 * kp:2 * kp + 2,
                                                    to * P:(to + 1) * P],
                                        rhs=wve[:, 2 * kp:2 * kp + 2,
                                                nh * 512:(nh + 1) * 512],
                                        perf_mode=DR,
                                        start=(kp == 0),
                                        stop=(kp == DO // 2 - 1))
                                nc.vector.tensor_scalar_mul(
                                    v65e[:, to, nh * 8:(nh + 1) * 8, 0:64],
                                    pv.rearrange("p (h d) -> p h d", d=HD),
                                    vsc)
                            else:
                                for ko in range(DO):
                                    nc.tensor.matmul(
                                        pv[:],
                                        lhsT=enc_sb[:, ko, to * P:(to + 1) * P],
                                        rhs=wve[:, ko, nh * 512:(nh + 1) * 512],
                                        start=(ko == 0), stop=(ko == DO - 1))
                                nc.vector.tensor_copy(
                                    v65e[:, to, nh * 8:(nh + 1) * 8, 0:64],
                                    pv.rearrange("p (h d) -> p h d", d=HD))
                        return fill

                    for do in range(DO):
                        fillers.append(mk_kenc(do))
                    for to in range(TO):
                        fillers.append(mk_venc(to, 0))
                    for to in range(TO):
                        fillers.append(mk_venc(to, 1))
                    ln2_fillers = fillers[16 - LN2_FILL:]
                    fillers = fillers[:16 - LN2_FILL]

                    if FILLERS:
                        attention(qt, kt, v65, att, True, f"sabv{l}", fillers)
                        proj_O(f"sao{l}", f"sabo{l}", att)
                        layer_norm(3 * l + 1, hb, fillers=ln2_fillers)
                    else:
                        attention(qt, kt, v65, att, True, f"sabv{l}")
                        proj_O(f"sao{l}", f"sabo{l}", att)
                        for fl in fillers + ln2_fillers:
                            fl()
                        layer_norm(3 * l + 1, hb)
                    proj_T(f"caq{l}", f"cabq{l}", hb, qt)
                    attention(qt, kte, v65e, att, False, f"cabv{l}",
                              dr=FP8_CROSS_AV)
                    proj_O(f"cao{l}", f"cabo{l}", att)

                    # ---- FFN ----
                    layer_norm(3 * l + 2, hbf)
                    b1off = boffs[f"b1{l}"]
                    for q in range(4):
                        w1q = load_w1(l, q)
                        for fl in range(8):
                            fo = q * 8 + fl
                            pf = psum()
                            for ko in range(DO):
                                nc.tensor.matmul(
                                    pf[:], lhsT=w1q[:, ko, fl * P:(fl + 1) * P],
                                    rhs=hbf[:, ko, :],
                                    start=(ko == 0), stop=(ko == DO - 1))
                            nc.scalar.activation(
                                uT[:, fo, :], pf[:], AF.Relu,
                                bias=bias_sb[:, b1off + fo:b1off + fo + 1])
                    b2off = boffs[f"b2{l}"]
                    for grp in range(2):
                        pys = [psum() for _ in range(4)]
                        for q in range(4):
                            w2q = load_w2(l, q)
                            for dl in range(4):
                                do = grp * 4 + dl
                                for kl in range(8):
                                    fo = q * 8 + kl
                                    nc.tensor.matmul(
                                        pys[dl][:],
                                        lhsT=w2q[:, kl, do * P:(do + 1) * P],
                                        rhs=uT[:, fo, :],
                                        start=(q == 0 and kl == 0),
                                        stop=(q == 3 and kl == 7))
                        for dl in range(4):
                            do = grp * 4 + dl
                            nc.vector.scalar_tensor_tensor(
                                x[:, do, :], pys[dl][:],
                                bias_sb[:, b2off + do:b2off + do + 1],
                                x[:, do, :], op0=OP.add, op1=OP.add)

                # ---- final LN + store ----
                out_sb = act.tile([P, DO, T], F32, tag="uT", name="osb")
                layer_norm(NSLOT - 1, out_sb, final=True)
                nc.sync.dma_start(out_dram[:], out_sb[:])

            for _ in range(repeat):
                body()

    _split_sync_waits(nc)
    return nc


# ------------------------------------------------------------------ entry ---

def kernel(**inputs):
    from concourse.bass_utils import run_bass_kernel_spmd

    nc = build_decoder(repeat=1)
    in_maps = prep_inputs(inputs)
    res = run_bass_kernel_spmd(nc, in_maps, core_ids=list(range(N_CORES)),
                               trace=False)
    return unshard(res.results)



# revision 29
# speedup vs baseline: 1.0142x; 1.0142x over previous
"""Trainium2 Bass kernel for nn_Decoder_64201171141372.

6-layer pre-norm transformer decoder (D=1024, H=16, F=4096, B=8, S=512).
Sharding: data-parallel over batch — each of the 8 NeuronCores computes one
batch element end-to-end; no collectives.

Device-side layout: activations are kept TRANSPOSED in SBUF as [d_part=128,
d_outer, t] so every linear layer runs as matmul(lhsT=W_tile, rhs=xT) with
the contraction dim on partitions. LayerNorm / softmax statistics (which are
per-token, i.e. per free-element) are computed with ones-matmuls and
broadcast back across partitions with rank-1 matmuls. All matmul operands
are bf16 (fp32 PSUM accumulation); the residual stream stays fp32.
"""

import math

import numpy as np
import ml_dtypes

import concourse.bass as bass
import concourse.mybir as mybir
import concourse.tile as tile
from concourse.masks import make_identity
from concourse.vector_clock import ScopedClock, VectorClock

bf16 = ml_dtypes.bfloat16
f8e4 = ml_dtypes.float8_e4m3fn
F32 = mybir.dt.float32
B16 = mybir.dt.bfloat16
F8 = mybir.dt.float8e4
I32 = mybir.dt.int32
DR = mybir.MatmulPerfMode.DoubleRow

P = 128
V, D, H, F, L = 32000, 1024, 16, 4096, 6
T = 512            # decoder seq len == encoder seq len
HD = D // H        # 64
DO = D // P        # 8
FO = F // P        # 32
TO = T // P        # 4
NSLOT = 3 * L + 1  # layernorm slots (3 per layer + final)
EPS = 1e-5
N_CORES = 8
N_PROCS = 27

AF = mybir.ActivationFunctionType
OP = mybir.AluOpType

PHASE_LOG = None   # set to a list to record (label, next-inst-id) phase marks


def _ph(nc, label):
    if PHASE_LOG is not None:
        PHASE_LOG.append((label, nc.next_id()))

# ---- tuning flags (overridable before build_decoder) ----
LN_PREP_ACT = False    # xbd/x2d copies on ACT (True) or DVE (False)
UB_ACT = True          # attention ub copy on ACT (True) or DVE (False)
CAUSAL_TRIM = True     # restrict causal blocks to un-masked q range
ATTN_PAIR = False      # pair-adjacent score matmuls
FILLERS = True         # interleave cross-K/V into self-attention
LN1_FILL = 2           # cross-K/V pieces used to fill the LN1 stats bubble
LN2_FILL = 2           # cross-K/V pieces held back to fill the LN2 stats bubble
ET_BUFS = 2            # expT pipeline depth
RB_BUFS = 2            # rdb/ub pipeline depth

# fp8 (e4m3) config: q/k/v projections tolerate fp8 (softmax damping / small
# contribution); o/w1/w2 measurably do not (rel err budget is 2e-2).
WS = 64.0              # fp8 weight pre-scale (keeps 0.02-scale weights normal)
FP8_QKV = True         # q,k,v projections (self+cross) via fp8 DoubleRow
FP8_CROSS_AV = True    # cross-attention AV via fp8 DoubleRow
FP8_LN_STATS = True    # LN stats matmuls via fp8 DoubleRow
FP8_FFN = False        # fp8 FFN exceeds the 2e-2 rel-err budget; keep bf16
FP8_W = ("saq", "sak", "sav", "caq", "cak", "cav")


class _TC(tile.TileContext):
    """TileContext whose exit drain splits sem waits one per instruction.

    The walrus build in this container only encodes a single sync-wait on
    CTRL-class (Drain/NoOp) instructions; the stock tile exit aggregates one
    wait per logical proc onto one Drain and dies in codegen with "Too many
    sync wait commands". Emit one single-wait NOP per proc on the SP queue
    instead; SP program order then makes the final drain safe with no waits.
    """

    def _drain_and_barrier(self, tick_clock, wait_clock):
        gc = tick_clock.global_clock
        for p in range(N_PROCS):
            t = gc[p]
            if t:
                nop = self.nc.sync.nop(nofuse=True)
                pc = VectorClock([t if q == p else 0 for q in range(N_PROCS)])
                wait_clock.add_sem_waits(nop.ins, ScopedClock({None: pc}))
        self.nc.sync.drain()
        self.nc.all_engine_barrier()
        assert self.sems is not None
        popped = self.nc._tile_sem_poison_stack.pop()
        assert popped is self._sem_poison
        self.nc.clear_and_free_semaphores(list(self.sems.allocated().values()))
        self.nc.all_engine_barrier()


def _split_sync_waits(nc, max_waits=1):
    """Walrus in this container encodes at most one sync-wait per instruction.

    Tile's wait assigner attaches one wait per depended-on logical proc; move
    the extras onto single-wait NoOps inserted just before the instruction on
    the same engine queue (program order preserves the sync semantics).
    """
    n_added = 0
    for f in nc.m.functions:
        for bb in f.blocks:
            insts = bb.instructions
            new_list = []
            for ins in insts:
                si = getattr(ins, "sync_info", None)
                waits = list(si.on_wait) if si is not None and si.on_wait else []
                if len(waits) > max_waits:
                    for w in waits[:-max_waits]:
                        nop = mybir.InstNoOp(
                            name=f"I-wsplit{n_added}", ins=[], outs=[])
                        nop.engine = ins.engine
                        nop.sync_info = mybir.SyncInfo(on_wait=[w], on_update=[])
                        new_list.append(nop)
                        n_added += 1
                    ins.sync_info = mybir.SyncInfo(
                        on_wait=waits[-max_waits:], on_update=list(si.on_update))
                new_list.append(ins)
            if n_added:
                insts[:] = new_list
    return n_added


# ---------------------------------------------------------------- packing ---

def _wpack_offsets():
    """Column offsets into the bf16 pack and the fp8 pack.

    Each weight W[din, dout] is stored as [128, din//128, dout] flattened on
    the free axis; value at [p, ko, n] = W[ko*128 + p, n]. Returns
    offs[name] = (kind, col, ko, n) with kind in {"w16", "w8"}, plus the two
    pack widths.
    """
    fp8 = set(FP8_W) if FP8_QKV else set()
    if FP8_FFN:
        fp8 |= {"w1", "w2"}
    offs = {}
    c16 = 0
    c8 = 0

    def add(name, nm, ko, n):
        nonlocal c16, c8
        if nm in fp8:
            offs[name] = ("w8", c8, ko, n)
            c8 += ko * n
        else:
            offs[name] = ("w16", c16, ko, n)
            c16 += ko * n

    for l in range(L):
        for nm, ko, n in [
            ("saq", 8, 1024), ("sak", 8, 1024), ("sav", 8, 1024), ("sao", 8, 1024),
            ("caq", 8, 1024), ("cak", 8, 1024), ("cav", 8, 1024), ("cao", 8, 1024),
            ("w1", 8, 4096), ("w2", 32, 1024),
        ]:
            add(f"{nm}{l}", nm, ko, n)
    return offs, c16, c8


def _bpack_offsets():
    """Column offsets into bpack [128, cols] f32: bias b[d] at [d%128, off + d//128]."""
    offs = {}
    c = 0

    def add(name, w):
        nonlocal c
        offs[name] = c
        c += w

    for l in range(L):
        for nm, w in [
            ("sabq", 8), ("sabk", 8), ("sabv", 8), ("sabo", 8),
            ("cabq", 8), ("cabk", 8), ("cabv", 8), ("cabo", 8),
            ("b1", 32), ("b2", 8),
        ]:
            add(f"{nm}{l}", w)
    return offs, c


def _col_major(w):
    """[din, n] -> [128, din//128, n] with [p, ko, n] = w[ko*128+p, n]."""
    din, n = w.shape
    return np.ascontiguousarray(w.reshape(din // P, P, n).transpose(1, 0, 2))


def _part_cols(b):
    """[d] -> [128, d//128] with [p, o] = b[o*128+p]."""
    return np.ascontiguousarray(b.reshape(-1, P).T)


def prep_inputs(inputs):
    """Host-side packing: returns (in_maps list for 8 cores)."""
    enc = np.asarray(inputs["encoder_output"], np.float32)       # [8, 512, 1024]
    dec = np.asarray(inputs["decoder_input"]).astype(np.int32)   # [8, 512]
    table = np.ascontiguousarray(np.asarray(inputs["embed_table"], np.float32))
    sa_w = np.asarray(inputs["sa_w"], np.float32)
    sa_b = np.asarray(inputs["sa_b"], np.float32)
    ca_w = np.asarray(inputs["ca_w"], np.float32)
    ca_b = np.asarray(inputs["ca_b"], np.float32)
    w1 = np.asarray(inputs["ffn_w1"], np.float32)
    b1 = np.asarray(inputs["ffn_b1"], np.float32)
    w2 = np.asarray(inputs["ffn_w2"], np.float32)
    b2 = np.asarray(inputs["ffn_b2"], np.float32)
    ln_g = np.asarray(inputs["ln_g"], np.float32)
    ln_b = np.asarray(inputs["ln_b"], np.float32)
    fin_g = np.asarray(inputs["final_g"], np.float32)
    fin_b = np.asarray(inputs["final_b"], np.float32)

    # fold the V-projection bias into the O-projection bias (exact:
    # softmax rows sum to 1, so (att + b_v) @ W_o + b_o = att @ W_o + b_o'
    # with b_o' = b_v @ W_o + b_o).
    sa_bo = sa_b[:, 3] + np.einsum('lv,lvd->ld', sa_b[:, 2], sa_w[:, 3])
    ca_bo = ca_b[:, 3] + np.einsum('lv,lvd->ld', ca_b[:, 2], ca_w[:, 3])

    woffs, wcols16, wcols8 = _wpack_offsets()
    wpack = np.empty((P, wcols16), dtype=bf16)
    wpack8 = np.empty((P, max(wcols8, 1)), dtype=f8e4)

    def put(name, w):
        kind, off, ko, n = woffs[name]
        flat = _col_major(w).reshape(P, -1)
        if kind == "w8":
            wpack8[:, off:off + ko * n] = (flat * WS).astype(f8e4)
        else:
            wpack[:, off:off + ko * n] = flat.astype(bf16)

    for l in range(L):
        for j, nm in enumerate(["saq", "sak", "sav", "sao"]):
            put(f"{nm}{l}", sa_w[l, j])
        for j, nm in enumerate(["caq", "cak", "cav", "cao"]):
            put(f"{nm}{l}", ca_w[l, j])
        put(f"w1{l}", w1[l])
        put(f"w2{l}", w2[l])

    # biases of fp8-scaled projections are pre-multiplied by WS: the device
    # keeps q/k/v activations scaled by WS and folds 1/WS into the exp scale
    # (q.k) and the softmax-denominator broadcast row (v), so the psum->sbuf
    # copy is a plain add of the (scaled) bias.
    qks = WS if FP8_QKV else 1.0
    boffs, bcols = _bpack_offsets()
    bpack = np.zeros((P, bcols), dtype=np.float32)
    for l in range(L):
        bpack[:, boffs[f"sabq{l}"]:boffs[f"sabq{l}"] + 8] = _part_cols(sa_b[l, 0] * qks)
        bpack[:, boffs[f"sabk{l}"]:boffs[f"sabk{l}"] + 8] = _part_cols(sa_b[l, 1] * qks)
        bpack[:, boffs[f"sabo{l}"]:boffs[f"sabo{l}"] + 8] = _part_cols(sa_bo[l])
        bpack[:, boffs[f"cabq{l}"]:boffs[f"cabq{l}"] + 8] = _part_cols(ca_b[l, 0] * qks)
        bpack[:, boffs[f"cabk{l}"]:boffs[f"cabk{l}"] + 8] = _part_cols(ca_b[l, 1] * qks)
        bpack[:, boffs[f"cabo{l}"]:boffs[f"cabo{l}"] + 8] = _part_cols(ca_bo[l])
        b1s = WS if FP8_FFN else 1.0
        bpack[:, boffs[f"b1{l}"]:boffs[f"b1{l}"] + 32] = _part_cols(b1[l] * b1s)
        bpack[:, boffs[f"b2{l}"]:boffs[f"b2{l}"] + 8] = _part_cols(b2[l])

    gln = np.empty((2, NSLOT, D), dtype=bf16)
    for l in range(L):
        for s in range(3):
            gln[0, 3 * l + s] = ln_g[l, s].astype(bf16)
            gln[1, 3 * l + s] = ln_b[l, s].astype(bf16)
    gln[0, NSLOT - 1] = fin_g.astype(bf16)
    gln[1, NSLOT - 1] = fin_b.astype(bf16)

    # positional encoding, transposed layout [128, 8, 512] fp32
    pos = np.arange(T, dtype=np.float32)[:, None]
    div = np.exp(np.arange(0, D, 2, dtype=np.float32) * (-math.log(10000.0) / D))
    pe = np.zeros((T, D), dtype=np.float32)
    pe[:, 0::2] = np.sin(pos * div)
    pe[:, 1::2] = np.cos(pos * div)
    peT = np.ascontiguousarray(
        pe.T.reshape(DO, P, T).transpose(1, 0, 2)).astype(bf16)

    # with CAUSAL_TRIM only the diagonal [P, P] block is ever masked, and it
    # is the same lower-triangular(<=) pattern for every kto: [p, q] = p <= q
    cmask = (np.arange(P)[:, None] <= np.arange(P)[None, :]).astype(bf16)

    enc_dt = f8e4 if FP8_QKV else bf16
    in_maps = []
    for c in range(N_CORES):
        encT = np.ascontiguousarray(
            enc[c].T.reshape(DO, P, T).transpose(1, 0, 2)).astype(enc_dt)
        in_maps.append({
            "wpack": wpack,
            "wpack8": wpack8,
            "bpack": bpack,
            "gln": gln,
            "table": table,
            "idx": dec[c].copy(),
            "encT": encT,
            "peT": peT,
            "cmask": cmask,
        })
    return in_maps


def unshard(results):
    """Per-core outT [128, 8, 512] -> full [8, 512, 1024] fp32."""
    out = np.empty((N_CORES, T, D), dtype=np.float32)
    for c in range(N_CORES):
        arr = results[c]["out"]                       # [dp, do, t]
        out[c] = arr.transpose(2, 1, 0).reshape(T, D)  # [t, do*128+dp]
    return out


# ----------------------------------------------------------------- device ---

def build_decoder(repeat: int = 1):
    nc = bass.Bass(trn_type="TRN2")
    woffs, wcols16, wcols8 = _wpack_offsets()
    boffs, bcols = _bpack_offsets()
    enc_dt = F8 if FP8_QKV else B16

    w_dram = nc.dram_tensor("wpack", [P, wcols16], B16, kind="ExternalInput")
    w8_dram = nc.dram_tensor("wpack8", [P, max(wcols8, 1)], F8,
                             kind="ExternalInput")
    b_dram = nc.dram_tensor("bpack", [P, bcols], F32, kind="ExternalInput")
    gln_dram = nc.dram_tensor("gln", [2, NSLOT, D], B16, kind="ExternalInput")
    table = nc.dram_tensor("table", [V, D], F32, kind="ExternalInput")
    idx_dram = nc.dram_tensor("idx", [T], I32, kind="ExternalInput")
    enc_dram = nc.dram_tensor("encT", [P, DO, T], enc_dt, kind="ExternalInput")
    pe_dram = nc.dram_tensor("peT", [P, DO, T], B16, kind="ExternalInput")
    cm_dram = nc.dram_tensor("cmask", [P, P], B16, kind="ExternalInput")
    out_dram = nc.dram_tensor("out", [P, DO, T], F32, kind="ExternalOutput")

    with _TC(nc) as tc:
        with tc.tile_pool(name="pers", bufs=1) as pers, \
             tc.tile_pool(name="wp", bufs=2) as wp, \
             tc.tile_pool(name="act", bufs=1) as act, \
             tc.tile_pool(name="sc", bufs=2) as scp, \
             tc.tile_pool(name="sm", bufs=1) as sm, \
             tc.tile_pool(name="ps", bufs=1, space="PSUM") as psp:

            # ---- persistent state ----
            x = pers.tile([P, DO, T], F32)       # residual stream (transposed)
            ones = pers.tile([P, 1], B16)
            ones8 = pers.tile([P, 2, 16], F8)    # fp8 ones; 16B k-stride for dual-fp8 ldweights
            ident = pers.tile([P, P], F32)
            bias_sb = pers.tile([P, bcols], F32)
            enc_sb = pers.tile([P, DO, T], enc_dt)
            cm_sb = pers.tile([P, P], B16)
            idx_sb = pers.tile([P, TO], I32)

            zcol = pers.tile([P, 1], F32)    # zero bias column for ACT ops
            epsc = pers.tile([1, 1], F32)    # eps bias for the LN sqrt
            sA = pers.tile([1, T], B16)      # LN scale row (bf16 rhs for A-mm)
            sB = pers.tile([2, T], B16)      # LN shift row + ones row (B-mm rhs)
            wsinv = pers.tile([P, HD], B16)  # 1/WS rows for the denom bcast

            nc.vector.memset(wsinv[:], 1.0 / WS if FP8_QKV else 1.0)
            nc.vector.memset(ones[:], 1.0)
            nc.vector.memset(ones8[:], 1.0)
            nc.vector.memset(sB[:], 1.0)
            nc.vector.memset(zcol[:], 0.0)
            nc.vector.memset(epsc[:], EPS)
            make_identity(nc, ident[:])
            nc.sync.dma_start(bias_sb[:], b_dram[:])
            nc.sync.dma_start(enc_sb[:], enc_dram[:])
            nc.sync.dma_start(cm_sb[:], cm_dram[:])
            nc.sync.dma_start(idx_sb[:], idx_dram.rearrange("(ti p) -> p ti", p=P))

            def psum(tag="ps"):
                return psp.tile([P, T], F32, tag=tag, name="pt", bufs=4)

            def psumpp():
                return psp.tile([P, 2, T], F32, tag="pp", name="pt2", bufs=2)

            def load_w(name, tag=None):
                kind, off, ko, n = woffs[name]
                if n == 4096:  # w1: select a 1024-wide column group q later
                    raise AssertionError("use load_w1")
                if kind == "w8":
                    wt = wp.tile([P, 8, 1024], F8, tag=tag or "w8")
                    src = w8_dram[:, off:off + ko * n].rearrange(
                        "p (o n) -> p o n", o=ko)
                else:
                    wt = wp.tile([P, 8, 1024], B16, tag=tag or "w")
                    src = w_dram[:, off:off + ko * n].rearrange(
                        "p (o n) -> p o n", o=ko)
                nc.sync.dma_start(wt[:], src)
                return wt, kind

            def load_w1(l, q):
                kind, off, ko, n = woffs[f"w1{l}"]
                if kind == "w8":
                    wt = wp.tile([P, 8, 1024], F8, tag="w8")
                    src = w8_dram[:, off:off + ko * n].rearrange(
                        "p (o n) -> p o n", o=ko)
                else:
                    wt = wp.tile([P, 8, 1024], B16, tag="w")
                    src = w_dram[:, off:off + ko * n].rearrange(
                        "p (o n) -> p o n", o=ko)
                nc.sync.dma_start(wt[:], src[:, :, q * 1024:(q + 1) * 1024])
                return wt, kind

            def load_w2(l, q):
                kind, off, ko, n = woffs[f"w2{l}"]
                if kind == "w8":
                    wt = wp.tile([P, 8, 1024], F8, tag="w8")
                    src = w8_dram[:, off + q * 8192: off + (q + 1) * 8192]
                else:
                    wt = wp.tile([P, 8, 1024], B16, tag="w")
                    src = w_dram[:, off + q * 8192: off + (q + 1) * 8192]
                nc.sync.dma_start(wt[:], src.rearrange("p (o n) -> p o n", o=8))
                return wt, kind

            def mm_col(pq, wt, kind, do, rhs_t, q0=0):
                """Accumulate one output column tile: psum += W.T @ rhs."""
                if kind == "w8":
                    for kp in range(DO // 2):
                        nc.tensor.matmul(
                            pq[:, q0:],
                            lhsT=wt[:, 2 * kp:2 * kp + 2, do * P:(do + 1) * P],
                            rhs=rhs_t[:, 2 * kp:2 * kp + 2, q0:],
                            perf_mode=DR,
                            start=(kp == 0), stop=(kp == DO // 2 - 1))
                else:
                    for ko in range(DO):
                        nc.tensor.matmul(
                            pq[:, q0:], lhsT=wt[:, ko, do * P:(do + 1) * P],
                            rhs=rhs_t[:, ko, q0:],
                            start=(ko == 0), stop=(ko == DO - 1))

            # ---------------- layer building blocks ----------------
            def layer_norm(slot, out_t, final=False, fillers=()):
                """out_t[:, do, :] = LN(x) using gln[:, slot]; out dtype = out_t's."""
                gl = sm.tile([2, 1, D], B16, tag="gl")
                nc.sync.dma_start(gl[:], gln_dram[:, slot, :][:, None, :])
                s12 = psumpp()
                s1, s2 = s12[:, 0, :], s12[:, 1, :]
                if FP8_LN_STATS:
                    # pair-granular fp8 copies + DoubleRow stat matmuls.
                    # x2d holds x^2/8 (keeps squares in e4m3 range).
                    xbds, x2ds = [], []
                    for kp in range(DO // 2):
                        xbd = scp.tile([P, 2, T], F8, tag="xbd", bufs=4,
                                       name="xbd")
                        nc.vector.tensor_copy(
                            xbd[:], x[:, 2 * kp:2 * kp + 2, :])
                        xbds.append(xbd)
                    for kp in range(DO // 2):
                        x2d = scp.tile([P, 2, T], F8, tag="x2d", bufs=4,
                                       name="x2d")
                        nc.vector.scalar_tensor_tensor(
                            x2d[:], x[:, 2 * kp:2 * kp + 2, :], 0.125,
                            x[:, 2 * kp:2 * kp + 2, :],
                            op0=OP.mult, op1=OP.mult)
                        x2ds.append(x2d)
                    for kp in range(DO // 2):
                        nc.tensor.matmul(s1[0:2, :], lhsT=ones8[:, :, 0:2],
                                         rhs=xbds[kp][:], perf_mode=DR,
                                         start=(kp == 0),
                                         stop=(kp == DO // 2 - 1))
                        nc.tensor.matmul(s2[0:2, :], lhsT=ones8[:, :, 0:2],
                                         rhs=x2ds[kp][:], perf_mode=DR,
                                         start=(kp == 0),
                                         stop=(kp == DO // 2 - 1))
                    s2scale = 8.0
                else:
                    # group same-function ACT ops to avoid table thrash
                    for g in range(2):
                        xbds, x2ds = [], []
                        for dl in range(4):
                            xbd = scp.tile([P, T], B16, tag="xbd", bufs=4,
                                           name="xbd")
                            if LN_PREP_ACT:
                                nc.scalar.copy(xbd[:], x[:, g * 4 + dl, :])
                            else:
                                nc.vector.tensor_copy(xbd[:], x[:, g * 4 + dl, :])
                            xbds.append(xbd)
                        for dl in range(4):
                            x2d = scp.tile([P, T], B16, tag="x2d", bufs=4,
                                           name="x2d")
                            if LN_PREP_ACT:
                                nc.scalar.square(x2d[:], x[:, g * 4 + dl, :])
                            else:
                                nc.vector.tensor_tensor(
                                    x2d[:], x[:, g * 4 + dl, :],
                                    x[:, g * 4 + dl, :], op=OP.mult)
                            x2ds.append(x2d)
                        for dl in range(4):
                            do = g * 4 + dl
                            nc.tensor.matmul(s1[0:1, :], lhsT=ones[:, 0:1],
                                             rhs=xbds[dl][:],
                                             start=(do == 0), stop=(do == DO - 1))
                            nc.tensor.matmul(s2[0:1, :], lhsT=ones[:, 0:1],
                                             rhs=x2ds[dl][:],
                                             start=(do == 0), stop=(do == DO - 1))
                    s2scale = 1.0
                for f in fillers:
                    f()
                m = sm.tile([1, T], F32, tag="m")
                t1 = sm.tile([1, T], F32, tag="t1")
                t2 = sm.tile([1, T], F32, tag="t2")
                # mean; m^2; var = s2*s2scale/D - m^2; rstd = rsqrt(var+eps)
                nc.vector.tensor_scalar_mul(m[:], s1[0:1, :], 1.0 / D)
                nc.vector.tensor_tensor(t1[:], m[:], m[:], op=OP.mult)
                nc.vector.scalar_tensor_tensor(t2[:], s2[0:1, :], s2scale / D,
                                               t1[:],
                                               op0=OP.mult, op1=OP.subtract)
                nc.scalar.activation(t1[:], t2[:], AF.Sqrt, bias=epsc[:])
                nc.vector.reciprocal(t2[:], t1[:])
                nc.vector.tensor_copy(sA[:], t2[:])
                nc.vector.scalar_tensor_tensor(sB[0:1, :], m[:], -1.0, t2[:],
                                               op0=OP.mult, op1=OP.mult)
                for do in range(DO):
                    AB = psumpp()
                    A, Bp = AB[:, 0, :], AB[:, 1, :]
                    nc.tensor.matmul(A[:], lhsT=gl[0:1, 0, do * P:(do + 1) * P],
                                     rhs=sA[:], start=True, stop=True)
                    nc.tensor.matmul(Bp[:], lhsT=gl[0:2, 0, do * P:(do + 1) * P],
                                     rhs=sB[:], start=True, stop=True)
                    tmp = scp.tile([P, T], B16, tag="tmp")
                    nc.vector.tensor_tensor(tmp[:], x[:, do, :], A[:], op=OP.mult)
                    nc.vector.tensor_tensor(out_t[:, do, :], tmp[:], Bp[:], op=OP.add)
                    if final:
                        nc.sync.dma_start(out_dram[:, do, :], out_t[:, do, :])

            def proj_T(wname, bname, rhs_t, out_t, ko_outer=False):
                """out_t[dout, t] (transposed layout, bf16) = W.T @ rhs + b.

                ko_outer: iterate the contraction dim outermost (groups of 4
                output tiles) so the first matmuls only need rhs slice ko=0 —
                used for the first consumer after a layernorm, whose apply
                produces rhs slices incrementally."""
                wt, kind = load_w(wname)
                boff = boffs[bname]
                if not ko_outer:
                    for do in range(DO):
                        pq = psum()
                        mm_col(pq, wt, kind, do, rhs_t)
                        nc.scalar.activation(
                            out_t[:, do, :], pq[:], AF.Identity,
                            bias=bias_sb[:, boff + do:boff + do + 1])
                else:
                    for grp in range(2):
                        pqs = [psum() for _ in range(4)]
                        if kind == "w8":
                            for kp in range(DO // 2):
                                for dl in range(4):
                                    do = grp * 4 + dl
                                    nc.tensor.matmul(
                                        pqs[dl][:],
                                        lhsT=wt[:, 2 * kp:2 * kp + 2,
                                                do * P:(do + 1) * P],
                                        rhs=rhs_t[:, 2 * kp:2 * kp + 2, :],
                                        perf_mode=DR,
                                        start=(kp == 0), stop=(kp == DO // 2 - 1))
                        else:
                            for ko in range(DO):
                                for dl in range(4):
                                    do = grp * 4 + dl
                                    nc.tensor.matmul(
                                        pqs[dl][:], lhsT=wt[:, ko, do * P:(do + 1) * P],
                                        rhs=rhs_t[:, ko, :],
                                        start=(ko == 0), stop=(ko == DO - 1))
                        for dl in range(4):
                            do = grp * 4 + dl
                            nc.scalar.activation(
                                out_t[:, do, :], pqs[dl][:], AF.Identity,
                                bias=bias_sb[:, boff + do:boff + do + 1])

            def proj_V(wname, rhs_t, v65_t):
                """v65_t[:, to, h, 0:64] = (rhs.T @ Wv) in natural [t, dout] layout."""
                wt, kind = load_w(wname)
                for to in range(TO):
                    for nh in range(2):
                        pv = psum()
                        if kind == "w8":
                            for kp in range(DO // 2):
                                nc.tensor.matmul(
                                    pv[:],
                                    lhsT=rhs_t[:, 2 * kp:2 * kp + 2,
                                               to * P:(to + 1) * P],
                                    rhs=wt[:, 2 * kp:2 * kp + 2,
                                           nh * 512:(nh + 1) * 512],
                                    perf_mode=DR,
                                    start=(kp == 0), stop=(kp == DO // 2 - 1))
                        else:
                            for ko in range(DO):
                                nc.tensor.matmul(
                                    pv[:], lhsT=rhs_t[:, ko, to * P:(to + 1) * P],
                                    rhs=wt[:, ko, nh * 512:(nh + 1) * 512],
                                    start=(ko == 0), stop=(ko == DO - 1))
                        nc.scalar.activation(
                            v65_t[:, to, nh * 8:(nh + 1) * 8, 0:64],
                            pv.rearrange("p (h d) -> p h d", d=HD), AF.Copy)

            def attention(qt_t, kt_t, v65_t, out_att, causal,
                          fillers=(), dr=False):
                """Pipelined per-head (or per-pair) softmax attention.

                q/k/v carry a WS scale when FP8_QKV: scores are WS^2 too big
                (folded into the exp scale) and the AV sums are WS too big
                (folded into the 1/WS denominator-broadcast row).

                dr: et in fp8 + DoubleRow AV (cross-attention only — needs
                the full untrimmed q range per k-tile)."""
                fillers = list(fillers)
                trim = causal and CAUSAL_TRIM
                et_dt = F8 if dr else B16
                exp_scale = 1.0 / (math.sqrt(HD) * (WS * WS if FP8_QKV else 1.0))
                rb_row = wsinv[HD:HD + 1, 0:HD]  # memset 1/WS (1.0 if no fp8)
                assert not (dr and (ATTN_PAIR or causal))

                def q0_of(kto):
                    return kto * P if trim else 0

                def scores_exp(h):
                    """scores + exp (+mask) for one head; returns et.

                    Scores land in 2-bank pair tiles; the non-causal path
                    exps a whole pair in one ACT op."""
                    base = (h % 2) * HD
                    doh = h // 2
                    et = scp.tile([P, TO, T], et_dt, tag="expT", bufs=ET_BUFS,
                                  name="et")
                    for half in range(2):
                        pp = psumpp()
                        for j in range(2):
                            kto = 2 * half + j
                            q0 = q0_of(kto)
                            nc.tensor.matmul(
                                pp[:, j, q0:],
                                lhsT=kt_t[base:base + HD, doh,
                                          kto * P:(kto + 1) * P],
                                rhs=qt_t[base:base + HD, doh, q0:],
                                start=True, stop=True)
                        if not causal:
                            nc.scalar.activation(
                                et[:, 2 * half:2 * half + 2, :], pp[:, :, :],
                                AF.Exp, bias=zcol[:], scale=exp_scale)
                            continue
                        for j in range(2):
                            kto = 2 * half + j
                            q0 = q0_of(kto)
                            nc.scalar.activation(et[:, kto, q0:],
                                                 pp[:, j, q0:],
                                                 AF.Exp, bias=zcol[:],
                                                 scale=exp_scale)
                            qe = q0 + P if trim else T
                            nc.vector.tensor_tensor(
                                et[:, kto, q0:qe], et[:, kto, q0:qe],
                                cm_sb[:, 0:qe - q0], op=OP.mult)
                    return et

                def pair_scores_exp(pr):
                    """scores + exp for a head pair, score mms pair-adjacent."""
                    et = scp.tile([P, TO, 2, T], B16, tag="expT", bufs=2,
                                  name="et")
                    for kto in range(TO):
                        q0 = q0_of(kto)
                        scs = []
                        for e in range(2):
                            sc = psum()
                            nc.tensor.matmul(
                                sc[:, q0:],
                                lhsT=kt_t[e * HD:(e + 1) * HD, pr,
                                          kto * P:(kto + 1) * P],
                                rhs=qt_t[e * HD:(e + 1) * HD, pr, q0:],
                                start=True, stop=True)
                            scs.append(sc)
                        for e in range(2):
                            nc.scalar.activation(et[:, kto, e, q0:],
                                                 scs[e][:, q0:], AF.Exp,
                                                 bias=zcol[:],
                                                 scale=exp_scale)
                        if causal:
                            qe = q0 + P if trim else T
                            for e in range(2):
                                nc.vector.tensor_tensor(
                                    et[:, kto, e, q0:qe], et[:, kto, e, q0:qe],
                                    cm_sb[:, 0:qe - q0], op=OP.mult)
                    return et

                def emit_ud(h, et_sl, et_tile=None):
                    ud = psum()
                    if dr:
                        # fp8 DoubleRow over k-tile pairs (full q range)
                        for kp in range(TO // 2):
                            nc.tensor.matmul(
                                ud[0:HD + 1, :],
                                lhsT=v65_t[:, 2 * kp:2 * kp + 2, h, :],
                                rhs=et_tile[:, 2 * kp:2 * kp + 2, :],
                                perf_mode=DR,
                                start=(kp == 0), stop=(kp == TO // 2 - 1))
                        return ud
                    for kto in range(TO):
                        q0 = q0_of(kto)
                        nc.tensor.matmul(ud[0:HD + 1, q0:],
                                         lhsT=v65_t[:, kto, h, :],
                                         rhs=et_sl(kto)[:, q0:],
                                         start=(kto == 0), stop=(kto == TO - 1))
                    return ud

                def emit_recip_ub(ud):
                    rdb = scp.tile([P, T], B16, tag="rdb", bufs=RB_BUFS, name="rdb")
                    with nc.allow_low_precision("softmax denom recip bf16"):
                        nc.vector.reciprocal(rdb[HD:HD + 1, :],
                                             ud[HD:HD + 1, :])
                    ub = scp.tile([P, T], B16, tag="ub", bufs=RB_BUFS, name="ub")
                    if causal:   # self: ACT free-ish; cross: ACT is exp-bound
                        nc.scalar.activation(ub[0:HD, :], ud[0:HD, :], AF.Copy)
                    else:
                        nc.vector.tensor_copy(ub[0:HD, :], ud[0:HD, :])
                    return rdb, ub

                def emit_norm(h, ub, rdb):
                    base = (h % 2) * HD
                    doh = h // 2
                    rb = psum()
                    nc.tensor.matmul(rb[0:HD, :], lhsT=rb_row,
                                     rhs=rdb[HD:HD + 1, :], start=True, stop=True)
                    sl = out_att[base:base + HD, doh, :]
                    nc.vector.tensor_tensor(sl, ub[0:HD, :], rb[0:HD, :],
                                            op=OP.mult)

                if not ATTN_PAIR:
                    et = scores_exp(0)
                    pending = None
                    for h in range(H):
                        if h + 1 < H:
                            net = scores_exp(h + 1)
                        cur = et
                        ud = emit_ud(h, lambda kto: cur[:, kto, :], et_tile=cur)
                        rdb, ub = emit_recip_ub(ud)
                        if fillers:
                            fillers.pop(0)()
                        if pending is not None:
                            emit_norm(*pending)
                        pending = (h, ub, rdb)
                        if h + 1 < H:
                            et = net
                    emit_norm(*pending)
                else:
                    et = pair_scores_exp(0)
                    for pr in range(H // 2):
                        cur = et
                        items = []
                        for e in range(2):
                            ud = emit_ud(2 * pr + e,
                                         lambda kto, e=e: cur[:, kto, e, :])
                            rdb, ub = emit_recip_ub(ud)
                            items.append((2 * pr + e, ub, rdb))
                        if fillers:
                            fillers.pop(0)()
                        if fillers:
                            fillers.pop(0)()
                        for it in items:
                            emit_norm(*it)
                        if pr + 1 < H // 2:
                            et = pair_scores_exp(pr + 1)
                for f in fillers:
                    f()

            def proj_O(wname, bname, rhs_att):
                """x += W.T @ att + b (residual update)."""
                wt, kind = load_w(wname)
                assert kind == "w16", "o-projection stays bf16 (error budget)"
                boff = boffs[bname]
                for do in range(DO):
                    po = psum()
                    for ko in range(DO):
                        nc.tensor.matmul(po[:], lhsT=wt[:, ko, do * P:(do + 1) * P],
                                         rhs=rhs_att[:, ko, :],
                                         start=(ko == 0), stop=(ko == DO - 1))
                    nc.vector.scalar_tensor_tensor(
                        x[:, do, :], po[:], bias_sb[:, boff + do:boff + do + 1],
                        x[:, do, :], op0=OP.add, op1=OP.add)

            # ---------------- full forward pass ----------------
            def body():
                _ph(nc, 'embed')
                # embedding: gather rows, transpose via PE, scale + pos-enc
                for ti in range(TO):
                    x0 = scp.tile([P, D], F32, tag="x0", bufs=1)
                    nc.gpsimd.indirect_dma_start(
                        out=x0[:], out_offset=None, in_=table[:],
                        in_offset=bass.IndirectOffsetOnAxis(
                            ap=idx_sb[:, ti:ti + 1], axis=0))
                    for do in range(DO):
                        pst = psum()
                        nc.tensor.transpose(pst[:, 0:P], x0[:, do * P:(do + 1) * P],
                                            ident[:])
                        pe_part = scp.tile([P, P], B16, tag="pe", bufs=2)
                        nc.sync.dma_start(pe_part[:],
                                          pe_dram[:, do, ti * P:(ti + 1) * P])
                        nc.vector.scalar_tensor_tensor(
                            x[:, do, ti * P:(ti + 1) * P], pst[:, 0:P],
                            math.sqrt(D), pe_part[:], op0=OP.mult, op1=OP.add)

                hdt = F8 if FP8_QKV else B16
                vedt = F8 if FP8_CROSS_AV else B16
                hb = act.tile([P, DO, T], hdt, tag="hb")      # attention LNs
                fdt = F8 if FP8_FFN else B16
                hbf = act.tile([P, DO, T], fdt, tag="hbf", name="hbf")
                qt = act.tile([P, DO, T], B16, tag="qt")
                kt = act.tile([P, DO, T], B16, tag="kt", bufs=2)

                for l in range(L):
                    _ph(nc, f'L{l}.start')
                    att = act.tile([P, DO, T], B16, tag="att", name="att")
                    # cross-attn K/V only depend on the encoder: emit them as
                    # fillers inside ln1 stats bubbles and between
                    # self-attention heads to keep PE busy. Own weight-ring
                    # tag so the loads don't serialize against saq/sak/sav.
                    kte = act.tile([P, DO, T], B16, tag="kt", bufs=2)
                    v65e = act.tile([P, TO, H, HD + 1], vedt, tag="v65",
                                    bufs=2)
                    wke, kindk = load_w(f"cak{l}", tag="w")
                    wve, kindv = load_w(f"cav{l}", tag="w")
                    kboff = boffs[f"cabk{l}"]
                    fillers = []

                    def mk_kenc(do, wke=wke, kindk=kindk, kte=kte,
                                kboff=kboff):
                        def fill():
                            pq = psum()
                            mm_col(pq, wke, kindk, do, enc_sb)
                            nc.vector.tensor_scalar_add(
                                kte[:, do, :], pq[:],
                                bias_sb[:, kboff + do:kboff + do + 1])
                        return fill

                    def mk_venc(to, nh, wve=wve, kindv=kindv, v65e=v65e):
                        def fill():
                            if to == 0 and nh == 0:
                                nc.vector.memset(v65e[:, :, :, HD:HD + 1], 1.0)
                            pv = psum()
                            if kindv == "w8":
                                for kp in range(DO // 2):
                                    nc.tensor.matmul(
                                        pv[:],
                                        lhsT=enc_sb[:, 2 * kp:2 * kp + 2,
                                                    to * P:(to + 1) * P],
                                        rhs=wve[:, 2 * kp:2 * kp + 2,
                                                nh * 512:(nh + 1) * 512],
                                        perf_mode=DR,
                                        start=(kp == 0),
                                        stop=(kp == DO // 2 - 1))
                            else:
                                for ko in range(DO):
                                    nc.tensor.matmul(
                                        pv[:],
                                        lhsT=enc_sb[:, ko, to * P:(to + 1) * P],
                                        rhs=wve[:, ko, nh * 512:(nh + 1) * 512],
                                        start=(ko == 0), stop=(ko == DO - 1))
                            nc.scalar.activation(
                                v65e[:, to, nh * 8:(nh + 1) * 8, 0:64],
                                pv.rearrange("p (h d) -> p h d", d=HD),
                                AF.Copy)
                        return fill

                    for do in range(DO):
                        fillers.append(mk_kenc(do))
                    for to in range(TO):
                        fillers.append(mk_venc(to, 0))
                    for to in range(TO):
                        fillers.append(mk_venc(to, 1))
                    ln1_fillers = fillers[:LN1_FILL]
                    ln2_fillers = fillers[16 - LN2_FILL:]
                    fillers = fillers[LN1_FILL:16 - LN2_FILL]

                    # ---- self attention ----
                    _ph(nc, f'L{l}.ln1')
                    layer_norm(3 * l + 0, hb,
                               fillers=ln1_fillers if FILLERS else ())
                    _ph(nc, f'L{l}.saqkv')
                    proj_T(f"saq{l}", f"sabq{l}", hb, qt)
                    proj_T(f"sak{l}", f"sabk{l}", hb, kt)
                    v65 = act.tile([P, TO, H, HD + 1], B16, tag="v65", bufs=2)
                    nc.vector.memset(v65[:, :, :, HD:HD + 1], 1.0)
                    proj_V(f"sav{l}", hb, v65)

                    if FILLERS:
                        _ph(nc, f'L{l}.sattn')
                        attention(qt, kt, v65, att, True, fillers)
                        _ph(nc, f'L{l}.sao')
                        proj_O(f"sao{l}", f"sabo{l}", att)
                        _ph(nc, f'L{l}.ln2')
                        layer_norm(3 * l + 1, hb, fillers=ln2_fillers)
                    else:
                        attention(qt, kt, v65, att, True)
                        proj_O(f"sao{l}", f"sabo{l}", att)
                        for fl in ln1_fillers + fillers + ln2_fillers:
                            fl()
                        layer_norm(3 * l + 1, hb)
                    _ph(nc, f'L{l}.caq')
                    proj_T(f"caq{l}", f"cabq{l}", hb, qt)
                    _ph(nc, f'L{l}.cattn')
                    attention(qt, kte, v65e, att, False,
                              dr=FP8_CROSS_AV)
                    _ph(nc, f'L{l}.cao')
                    proj_O(f"cao{l}", f"cabo{l}", att)

                    # ---- FFN ----
                    _ph(nc, f'L{l}.ln3')
                    layer_norm(3 * l + 2, hbf)
                    _ph(nc, f'L{l}.ffn1')
                    b1off = boffs[f"b1{l}"]
                    b2off = boffs[f"b2{l}"]
                    uT = act.tile([P, FO, T], B16, tag="uT", name="uT")
                    for q in range(4):
                        w1q, w1kind = load_w1(l, q)
                        for fl in range(8):
                            fo = q * 8 + fl
                            pf = psum()
                            for ko in range(DO):
                                nc.tensor.matmul(
                                    pf[:],
                                    lhsT=w1q[:, ko, fl * P:(fl + 1) * P],
                                    rhs=hbf[:, ko, :],
                                    start=(ko == 0), stop=(ko == DO - 1))
                            nc.scalar.activation(
                                uT[:, fo, :], pf[:], AF.Relu,
                                bias=bias_sb[:, b1off + fo:b1off + fo + 1])
                    _ph(nc, f'L{l}.ffn2')
                    pys = [t[:, j, :] for t in (psumpp(), psumpp())
                           for j in range(2)] + [psum() for _ in range(4)]
                    for q in range(4):
                        w2q, w2kind = load_w2(l, q)
                        for do in range(DO):
                            for kl in range(8):
                                fo = q * 8 + kl
                                nc.tensor.matmul(
                                    pys[do][:],
                                    lhsT=w2q[:, kl, do * P:(do + 1) * P],
                                    rhs=uT[:, fo, :],
                                    start=(q == 0 and kl == 0),
                                    stop=(q == 3 and kl == 7))
                    for do in range(DO):
                        nc.vector.scalar_tensor_tensor(
                            x[:, do, :], pys[do][:],
                            bias_sb[:, b2off + do:b2off + do + 1],
                            x[:, do, :], op0=OP.add, op1=OP.add)

                # ---- final LN + store ----
                out_sb = act.tile([P, DO, T], F32, tag="uT", name="osb")
                layer_norm(NSLOT - 1, out_sb, final=True)

            for _ in range(repeat):
                body()

    _split_sync_waits(nc)
    return nc


# ------------------------------------------------------------------ entry ---

def kernel(**inputs):
    from concourse.bass_utils import run_bass_kernel_spmd

    nc = build_decoder(repeat=1)
    in_maps = prep_inputs(inputs)
    res = run_bass_kernel_spmd(nc, in_maps, core_ids=list(range(N_CORES)),
                               trace=False)
    return unshard(res.results)

